# revision 12
# baseline (speedup 1.0000x reference)
"""Locally-connected 2D conv (unshared weights), VALID, stride 2 — Trainium2 Bass kernel.

Problem (hardcoded):
  x:       (16, 32, 113, 113) f32
  weights: (56, 56, 32, 3, 3, 64) f32   (H_out, W_out, C_in, kh, kw, C_out)
  bias:    (56, 56, 64) f32
  out:     (16, 64, 56, 56) f32
  out[b,o,u,v] = sum_{c,q,r} x[b,c,2u+q,2v+r] * weights[u,v,c,q,r,o] + bias[u,v,o]

Sharding: H_out split across 8 cores (7 output rows each); each core reads only
its 1/8 of the weight tensor (the dominant HBM traffic).

The kernel is DMA-bound (~174 GB/s/core payload ceiling, all 16 SDMA engines
saturated), so everything is about minimizing bytes:
  - weights and x are cast to fp16 on the host (tolerance 2e-2; fp16 keeps the
    error ~3e-4), halving the dominant stream,
  - x is packed as the 7 even row-slabs actually read (3 q-shifted copies on
    96 partitions), not whole rows (the f32 baseline DMAed 13 slabs of which
    6 were never read),
  - bias rides as a 97th contraction row (ones row in x's stationary tile,
    bias values in the weight stream at the r=1 tap) - no bias DMA, no
    broadcast add,
  - output is written back fp16 (|out| <= ~90, ulp 0.06) and cast on host.

DMA layout rules learned from traces: a DMA whose partition count is not a
multiple of 16 is NOT sprayed across the 16 SDMA engines (it lands on one
engine at ~26 GB/s), so every 97-row transfer is split into a 96-row DMA plus
a 1-row DMA. Weight tiles ride the SP HWDGE ring in consumption order;
x/outputs ride the ACT ring so a pending output DMA never FIFO-blocks a
weight prefetch.

Per-core compute: x is the PE *stationary* operand (LDWEIGHTS of 16 columns)
and the weights are the *moving* operand: for input column w, the taps
(v, r=w-2v) consume weight columns [v*192+r*64, +64) which are contiguous, so
an even w does one N=128 matmul for both taps. PSUM accumulates f32 in
one-bank chunks of 8 output columns ([16, 512] f32 = 2KB = the PSUM
zero-region granularity; start=True only arms pending-zero for the 2KB region
containing the first matmul's target, so a chunk must not span banks). The
DVE copies each finished chunk to fp16 SBUF staging; one DMA per u streams it
out.
"""

import numpy as np

B = 16
C_IN = 32
C_OUT = 64
H_OUT = 56
W_OUT = 56
KK = 3
STRIDE = 2
H_IN = 113

N_CORES = 8
U_PER = H_OUT // N_CORES          # 7 output rows per core
T_ROWS = U_PER                    # 7 even row-slabs per q-shifted copy
VCHUNK = 8                        # output cols per PSUM chunk (1 bank)
NCHUNK = W_OUT // VCHUNK          # 7 chunks per u
KPART = C_IN * KK + 1             # 96 contraction partitions (q,c) + bias row
XFREE = T_ROWS * H_IN * B         # x' tile free size (12656 fp16 elems)
WFREE = W_OUT * KK * C_OUT        # weight tile free size per u (10752)
RO = KK * C_OUT                   # 192: cols per v in the weight stream

_CACHE = {}


def _chunk_matmuls(ch):
    """Matmuls for one 8-v psum chunk: (w, psum_off_f32, col_off, ncols).

    psum offsets are f32 elements relative to the [16, 512] chunk tile; col
    offsets are relative to the per-u weight tile. The two taps of an even w
    fuse into one N=128 matmul when both fall in this chunk.

    PSUM has_written is per-byte and each matmul must be uniformly
    first-write or accumulate, so the odd-w (r=1) matmuls run first - each is
    the unique first writer of its 64-col v region - and every even-w matmul
    (fused or not) then purely accumulates.
    """
    v0 = ch * VCHUNK
    odd, even = [], []
    for w in range(2 * v0, 2 * (v0 + VCHUNK - 1) + 3):
        pairs = []
        for r in (2, 1, 0):
            v = (w - r) // 2
            if 2 * v + r == w and v0 <= v < v0 + VCHUNK and 0 <= v < W_OUT:
                pairs.append((v, r))
        if not pairs:
            continue
        if len(pairs) == 2:
            v, r = pairs[0]
            even.append((w, (v - v0) * C_OUT, v * RO + r * C_OUT, 2 * C_OUT))
        else:
            for v, r in pairs:
                dst = odd if w % 2 else even
                dst.append((w, (v - v0) * C_OUT, v * RO + r * C_OUT, C_OUT))
    return odd + even


def _build():
    import concourse.mybir as mybir
    from concourse import bacc
    from concourse.tile import TileContext

    f16 = mybir.dt.float16
    nc = bacc.Bacc("TRN2", target_bir_lowering=False, debug=False,
                   num_devices=N_CORES)
    # Host-prepacked tensors (see _pack_core):
    #   xp[p, t*113*16 + w*16 + b] = x[b, c, 2u0+q+2t, w],  p = q*32+c; row 96 = 1.0
    #   wp[u, p, v*192 + r*64 + o] = weights[u0+u, v, c, q, r, o];
    #     row 96 = bias[u0+u, v, o] at r==1, else 0
    #   y[u, b, v*64 + o] fp16
    xp_in = nc.dram_tensor("xp", [U_PER, KPART, H_IN * B], f16,
                           kind="ExternalInput").ap()
    wp_in = nc.dram_tensor("wp", [U_PER, KPART, WFREE], f16,
                           kind="ExternalInput").ap()
    y_out = nc.dram_tensor("y", [U_PER, B, W_OUT * C_OUT], f16,
                           kind="ExternalOutput").ap()

    with TileContext(nc) as tc:
        with tc.tile_pool(name="xpool", bufs=U_PER) as xpool, \
             tc.tile_pool(name="wpool", bufs=4) as wpool, \
             tc.tile_pool(name="w6pool", bufs=NCHUNK) as w6pool, \
             tc.tile_pool(name="opool", bufs=4) as opool, \
             tc.tile_pool(name="pspool", bufs=8, space="PSUM") as pspool:

            # x + outputs on the ACT ring; weights on the SP ring.
            # Partition counts on every dma_start are multiples of 16 (plus a
            # 1-row fixup) so the HWDGE sprays descriptors across all 16 SDMA
            # engines.
            # per-u x slabs: u0's 351KB lands in a few us so compute starts
            # early instead of waiting ~20us for one big x DMA
            xts = []
            for u in range(U_PER):
                xtu = xpool.tile([KPART, H_IN * B], f16)
                nc.scalar.dma_start(out=xtu[0:96, :], in_=xp_in[u, 0:96])
                nc.scalar.dma_start(out=xtu[96:97, :], in_=xp_in[u, 96:97])
                # (p, w, b) view for the stationary slices
                xts.append(xtu.rearrange("p (w b) -> p w b", b=B))

            # wp viewed per 8-v chunk so the last u can stream at chunk
            # granularity (tapers the pipeline-drain tail)
            wp4 = wp_in.rearrange("u p (ch f) -> u ch p f", ch=NCHUNK)
            CHF = WFREE // NCHUNK                     # 1536 cols per chunk
            for u in range(U_PER):
                last = u == U_PER - 1
                if not last:
                    wt = wpool.tile([KPART, WFREE], f16)
                    nc.sync.dma_start(out=wt[0:96, :], in_=wp_in[u, 0:96])
                    nc.sync.dma_start(out=wt[96:97, :], in_=wp_in[u, 96:97])

                stage = opool.tile([B, W_OUT * C_OUT], f16)
                for ch in range(NCHUNK):
                    if last:
                        wtc = w6pool.tile([KPART, CHF], f16)
                        nc.sync.dma_start(out=wtc[0:96, :],
                                          in_=wp4[u, ch, 0:96])
                        nc.sync.dma_start(out=wtc[96:97, :],
                                          in_=wp4[u, ch, 96:97])
                    ps = pspool.tile([B, VCHUNK * C_OUT], mybir.dt.float32)
                    mms = _chunk_matmuls(ch)
                    for i, (w, ps_off, col, ncol) in enumerate(mms):
                        lhsT = xts[u][:, w:w + 1, :]
                        src = (wtc[:, col - ch * CHF:col - ch * CHF + ncol]
                               if last else wt[:, col:col + ncol])
                        nc.tensor.matmul(
                            ps[:, ps_off:ps_off + ncol],
                            lhsT, src,
                            start=(i == 0), stop=(i == len(mms) - 1),
                        )
                    nc.vector.tensor_copy(
                        out=stage[:, ch * VCHUNK * C_OUT:
                                  (ch + 1) * VCHUNK * C_OUT],
                        in_=ps[:])
                nc.scalar.dma_start(out=y_out[u], in_=stage[:])

    nc.compile()
    return nc


def _get_nc():
    if "nc" not in _CACHE:
        _CACHE["nc"] = _build()
    return _CACHE["nc"]


def _pack_core(x16, w16, b16, i):
    u0 = i * U_PER
    # x': (7, 97, 113*16); xp[t, q*32+c] holds x[b, c, 2u0+q+2t, w] at (w, b)
    xs = x16[:, :, STRIDE * u0:STRIDE * u0 + 2 * U_PER + 1, :]  # (B,C,15,113)
    xq = np.stack([xs[:, :, q:q + 2 * U_PER - 1:STRIDE, :] for q in range(KK)],
                  axis=0)                                   # (q, b, c, t, w)
    xq = xq.transpose(3, 0, 2, 4, 1)                        # (t, q, c, w, b)
    xp = np.empty((U_PER, KPART, H_IN * B), dtype=np.float16)
    xp[:, :KPART - 1] = xq.reshape(U_PER, KPART - 1, H_IN * B)
    xp[:, KPART - 1] = np.float16(1.0)

    # w': (7, 97, 10752); p = q*32+c, free (v, r, o); row 96 = bias at r==1
    ws = w16[u0:u0 + U_PER]                             # (u, v, c, q, r, o)
    ws = ws.transpose(0, 3, 2, 1, 4, 5)                 # (u, q, c, v, r, o)
    wp = np.empty((U_PER, KPART, WFREE), dtype=np.float16)
    wp[:, :KPART - 1] = ws.reshape(U_PER, KPART - 1, WFREE)
    brow = np.zeros((U_PER, W_OUT, KK, C_OUT), dtype=np.float16)
    brow[:, :, 1, :] = b16[u0:u0 + U_PER]
    wp[:, KPART - 1] = brow.reshape(U_PER, WFREE)
    return {"xp": np.ascontiguousarray(xp), "wp": np.ascontiguousarray(wp)}


def kernel(x, weights, bias, _trace=False, _tmpdir=None):
    from concourse.bass_utils import run_bass_kernel_spmd

    x16 = np.asarray(x, dtype=np.float16)
    w16 = np.asarray(weights, dtype=np.float16)
    b16 = np.asarray(bias, dtype=np.float16)

    nc = _get_nc()
    core_ids = list(range(N_CORES))
    in_maps = [_pack_core(x16, w16, b16, i) for i in core_ids]
    res = run_bass_kernel_spmd(nc, in_maps, core_ids, trace=_trace,
                               tmpdir=_tmpdir)
    # y[u, b, v*64+o] per core -> out[b, o, u0+u, v]
    outs = []
    for i in core_ids:
        y = res.results[i]["y"].reshape(U_PER, B, W_OUT, C_OUT)
        outs.append(y.transpose(1, 3, 0, 2))
    out = np.concatenate(outs, axis=2).astype(np.float32)
    if _trace:
        _CACHE["last_result"] = res
    return out


# revision 15
# speedup vs baseline: 1.0310x; 1.0310x over previous
"""Locally-connected 2D conv (unshared weights), VALID, stride 2 — Trainium2 Bass kernel.

Problem (hardcoded):
  x:       (16, 32, 113, 113) f32
  weights: (56, 56, 32, 3, 3, 64) f32   (H_out, W_out, C_in, kh, kw, C_out)
  bias:    (56, 56, 64) f32
  out:     (16, 64, 56, 56) f32
  out[b,o,u,v] = sum_{c,q,r} x[b,c,2u+q,2v+r] * weights[u,v,c,q,r,o] + bias[u,v,o]

Sharding: H_out split across 8 cores (7 output rows each); each core reads only
its 1/8 of the weight tensor (the dominant HBM traffic).

The kernel is DMA-bound (~174 GB/s/core payload ceiling, all 16 SDMA engines
saturated), so everything is about minimizing bytes:
  - weights and x are cast to fp16 on the host (tolerance 2e-2; fp16 keeps the
    error ~3e-4), halving the dominant stream,
  - x is packed as the 7 even row-slabs actually read (3 q-shifted copies on
    96 partitions), not whole rows (the f32 baseline DMAed 13 slabs of which
    6 were never read),
  - bias rides as a 97th contraction row (ones row in x's stationary tile,
    bias values in the weight stream at the r=1 tap) - no bias DMA, no
    broadcast add,
  - output is written back fp16 (|out| <= ~90, ulp 0.06) and cast on host.

DMA layout rules learned from traces: a DMA whose partition count is not a
multiple of 16 is NOT sprayed across the 16 SDMA engines (it lands on one
engine at ~26 GB/s), so every 97-row transfer is split into a 96-row DMA plus
a 1-row DMA. Weight tiles ride the SP HWDGE ring in consumption order;
x/outputs ride the ACT ring so a pending output DMA never FIFO-blocks a
weight prefetch.

Per-core compute: x is the PE *stationary* operand (LDWEIGHTS of 16 columns)
and the weights are the *moving* operand: for input column w, the taps
(v, r=w-2v) consume weight columns [v*192+r*64, +64) which are contiguous, so
an even w does one N=128 matmul for both taps. PSUM accumulates f32 in
one-bank chunks of 8 output columns ([16, 512] f32 = 2KB = the PSUM
zero-region granularity; start=True only arms pending-zero for the 2KB region
containing the first matmul's target, so a chunk must not span banks). The
DVE copies each finished chunk to fp16 SBUF staging; one DMA per u streams it
out.
"""

import numpy as np

B = 16
C_IN = 32
C_OUT = 64
H_OUT = 56
W_OUT = 56
KK = 3
STRIDE = 2
H_IN = 113

N_CORES = 8
U_PER = H_OUT // N_CORES          # 7 output rows per core
T_ROWS = U_PER                    # 7 even row-slabs per q-shifted copy
VCHUNK = 8                        # output cols per PSUM chunk (1 bank)
NCHUNK = W_OUT // VCHUNK          # 7 chunks per u
KPART = C_IN * KK + 1             # 96 contraction partitions (q,c) + bias row
XFREE = T_ROWS * H_IN * B         # x' tile free size (12656 fp16 elems)
WFREE = W_OUT * KK * C_OUT        # weight tile free size per u (10752)
RO = KK * C_OUT                   # 192: cols per v in the weight stream

_CACHE = {}


def _chunk_matmuls(ch):
    """Matmuls for one 8-v psum chunk: (w, psum_off_f32, col_off, ncols).

    psum offsets are f32 elements relative to the [16, 512] chunk tile; col
    offsets are relative to the per-u weight tile. The two taps of an even w
    fuse into one N=128 matmul when both fall in this chunk.

    PSUM has_written is per-byte and each matmul must be uniformly
    first-write or accumulate, so the odd-w (r=1) matmuls run first - each is
    the unique first writer of its 64-col v region - and every even-w matmul
    (fused or not) then purely accumulates.
    """
    v0 = ch * VCHUNK
    odd, even = [], []
    for w in range(2 * v0, 2 * (v0 + VCHUNK - 1) + 3):
        pairs = []
        for r in (2, 1, 0):
            v = (w - r) // 2
            if 2 * v + r == w and v0 <= v < v0 + VCHUNK and 0 <= v < W_OUT:
                pairs.append((v, r))
        if not pairs:
            continue
        if len(pairs) == 2 and (pairs[0][0] + 1) % (W_OUT // 2) != 0:
            # fuse both taps unless the cols would span two half-u tiles
            v, r = pairs[0]
            even.append((w, (v - v0) * C_OUT, v * RO + r * C_OUT, 2 * C_OUT))
        else:
            for v, r in pairs:
                dst = odd if w % 2 else even
                dst.append((w, (v - v0) * C_OUT, v * RO + r * C_OUT, C_OUT))
    return odd + even


def _build():
    import concourse.mybir as mybir
    from concourse import bacc
    from concourse.tile import TileContext

    f16 = mybir.dt.float16
    nc = bacc.Bacc("TRN2", target_bir_lowering=False, debug=False,
                   num_devices=N_CORES)
    # Host-prepacked tensors (see _pack_core):
    #   xp[p, t*113*16 + w*16 + b] = x[b, c, 2u0+q+2t, w],  p = q*32+c; row 96 = 1.0
    #   wp[u, p, v*192 + r*64 + o] = weights[u0+u, v, c, q, r, o];
    #     row 96 = bias[u0+u, v, o] at r==1, else 0
    #   y[u, b, v*64 + o] fp16
    xp_in = nc.dram_tensor("xp", [U_PER, KPART, H_IN * B], f16,
                           kind="ExternalInput").ap()
    wp_in = nc.dram_tensor("wp", [U_PER, KPART, WFREE], f16,
                           kind="ExternalInput").ap()
    y_out = nc.dram_tensor("y", [U_PER, B, W_OUT * C_OUT], f16,
                           kind="ExternalOutput").ap()

    with TileContext(nc) as tc:
        with tc.tile_pool(name="xpool", bufs=U_PER) as xpool, \
             tc.tile_pool(name="wpool", bufs=8) as wpool, \
             tc.tile_pool(name="w6pool", bufs=NCHUNK) as w6pool, \
             tc.tile_pool(name="opool", bufs=4) as opool, \
             tc.tile_pool(name="pspool", bufs=8, space="PSUM") as pspool:

            # x + outputs on the ACT ring; weights on the SP ring.
            # Partition counts on every dma_start are multiples of 16 (plus a
            # 1-row fixup) so the HWDGE sprays descriptors across all 16 SDMA
            # engines.
            # All input DMAs ride ONE ring (SP) in exact consumption order:
            # xp0, w0a, w0b, xp1, w1a, w1b, ... so bytes arrive in the order
            # compute needs them at full aggregate DMA rate. Half-u weight
            # tiles keep PE idle gaps under the ~3.4us HAM re-throttle
            # window; the last u streams per 8-v chunk to shrink the
            # pipeline-drain tail. Outputs ride the ACT ring alone.
            HF = WFREE // 2                           # 5376 cols per half-u
            CHF = WFREE // NCHUNK                     # 1536 cols per chunk
            wp4 = wp_in.rearrange("u p (ch f) -> u ch p f", ch=NCHUNK)

            xts, wts = [], []
            for u in range(U_PER):
                xtu = xpool.tile([KPART, H_IN * B], f16)
                nc.sync.dma_start(out=xtu[0:96, :], in_=xp_in[u, 0:96])
                nc.sync.dma_start(out=xtu[96:97, :], in_=xp_in[u, 96:97])
                xts.append(xtu.rearrange("p (w b) -> p w b", b=B))
                if u < U_PER - 1:
                    halves = []
                    for h in range(2):
                        wh = wpool.tile([KPART, HF], f16)
                        sl = slice(h * HF, (h + 1) * HF)
                        nc.sync.dma_start(out=wh[0:96, :],
                                          in_=wp_in[u, 0:96, sl])
                        nc.sync.dma_start(out=wh[96:97, :],
                                          in_=wp_in[u, 96:97, sl])
                        halves.append(wh)
                    wts.append(halves)
                else:
                    chunks = []
                    for ch in range(NCHUNK):
                        wtc = w6pool.tile([KPART, CHF], f16)
                        nc.sync.dma_start(out=wtc[0:96, :],
                                          in_=wp4[u, ch, 0:96])
                        nc.sync.dma_start(out=wtc[96:97, :],
                                          in_=wp4[u, ch, 96:97])
                        chunks.append(wtc)
                    wts.append(chunks)

            for u in range(U_PER):
                last = u == U_PER - 1
                stage = opool.tile([B, W_OUT * C_OUT], f16)
                for ch in range(NCHUNK):
                    ps = pspool.tile([B, VCHUNK * C_OUT], mybir.dt.float32)
                    mms = _chunk_matmuls(ch)
                    for i, (w, ps_off, col, ncol) in enumerate(mms):
                        lhsT = xts[u][:, w:w + 1, :]
                        if last:
                            src = wts[u][ch][:, col - ch * CHF:
                                             col - ch * CHF + ncol]
                        else:
                            src = wts[u][col // HF][:, col % HF:
                                                    col % HF + ncol]
                        nc.tensor.matmul(
                            ps[:, ps_off:ps_off + ncol],
                            lhsT, src,
                            start=(i == 0), stop=(i == len(mms) - 1),
                        )
                    nc.vector.tensor_copy(
                        out=stage[:, ch * VCHUNK * C_OUT:
                                  (ch + 1) * VCHUNK * C_OUT],
                        in_=ps[:])
                nc.scalar.dma_start(out=y_out[u], in_=stage[:])

    nc.compile()
    return nc


def _get_nc():
    if "nc" not in _CACHE:
        _CACHE["nc"] = _build()
    return _CACHE["nc"]


def _pack_core(x16, w16, b16, i):
    u0 = i * U_PER
    # x': (7, 97, 113*16); xp[t, q*32+c] holds x[b, c, 2u0+q+2t, w] at (w, b)
    xs = x16[:, :, STRIDE * u0:STRIDE * u0 + 2 * U_PER + 1, :]  # (B,C,15,113)
    xq = np.stack([xs[:, :, q:q + 2 * U_PER - 1:STRIDE, :] for q in range(KK)],
                  axis=0)                                   # (q, b, c, t, w)
    xq = xq.transpose(3, 0, 2, 4, 1)                        # (t, q, c, w, b)
    xp = np.empty((U_PER, KPART, H_IN * B), dtype=np.float16)
    xp[:, :KPART - 1] = xq.reshape(U_PER, KPART - 1, H_IN * B)
    xp[:, KPART - 1] = np.float16(1.0)

    # w': (7, 97, 10752); p = q*32+c, free (v, r, o); row 96 = bias at r==1
    ws = w16[u0:u0 + U_PER]                             # (u, v, c, q, r, o)
    ws = ws.transpose(0, 3, 2, 1, 4, 5)                 # (u, q, c, v, r, o)
    wp = np.empty((U_PER, KPART, WFREE), dtype=np.float16)
    wp[:, :KPART - 1] = ws.reshape(U_PER, KPART - 1, WFREE)
    brow = np.zeros((U_PER, W_OUT, KK, C_OUT), dtype=np.float16)
    brow[:, :, 1, :] = b16[u0:u0 + U_PER]
    wp[:, KPART - 1] = brow.reshape(U_PER, WFREE)
    return {"xp": np.ascontiguousarray(xp), "wp": np.ascontiguousarray(wp)}


def kernel(x, weights, bias, _trace=False, _tmpdir=None):
    from concourse.bass_utils import run_bass_kernel_spmd

    x16 = np.asarray(x, dtype=np.float16)
    w16 = np.asarray(weights, dtype=np.float16)
    b16 = np.asarray(bias, dtype=np.float16)

    nc = _get_nc()
    core_ids = list(range(N_CORES))
    in_maps = [_pack_core(x16, w16, b16, i) for i in core_ids]
    res = run_bass_kernel_spmd(nc, in_maps, core_ids, trace=_trace,
                               tmpdir=_tmpdir)
    # y[u, b, v*64+o] per core -> out[b, o, u0+u, v]
    outs = []
    for i in core_ids:
        y = res.results[i]["y"].reshape(U_PER, B, W_OUT, C_OUT)
        outs.append(y.transpose(1, 3, 0, 2))
    out = np.concatenate(outs, axis=2).astype(np.float32)
    if _trace:
        _CACHE["last_result"] = res
    return out
